# revision 22
# baseline (speedup 1.0000x reference)
"""AFT-Full attention kernel for 8 Trainium2 NeuronCores.

Data-parallel over batch B=32 (4 batches per core). The cross-batch max
of K (used for exp stabilization) is computed with an AllReduce(max)
collective over the 8 cores, overlapped with the V/Q projections.

All matmuls run in bf16 with fp32 PSUM accumulation. Layout strategy:
  - x is transposed per batch (PE transposes) to xT[e,t] so the feature
    contraction of the K/V/Q projections sits on the partition axis.
  - K, V are produced in natural [t,d] layout; Q is produced transposed
    (QT[d,t]) and immediately turned into sigmoid(Q) (sigT).
  - num/den are computed transposed (numT[d,t] = U/expK contracted
    against exp_wT[s,t]) so the output projection consumes YtT[d,t]
    directly and writes out[t,e] in natural layout for the store.
"""
import os
import sys

sys.path.insert(0, '/opt/trn_rl_repo')
import numpy as np

B, T, D = 32, 1024, 512
N_CORES = 8
B_LOC = B // N_CORES           # 4 batches per core
NT = T // 128                  # 8 t-tiles
ND = D // 128                  # 4 d-tiles
NE = D // 128                  # 4 e-tiles
P = 128

_PHASE = int(os.environ.get("KPHASE", "9"))
_CACHED = None


class _PhaseStop(Exception):
    pass


def _build():
    from concourse import bacc, mybir, tile, masks

    f32 = mybir.dt.float32

    nc = bacc.Bacc("TRN2", target_bir_lowering=False, debug=False,
                   num_devices=N_CORES)

    io = {}
    io["x"] = nc.dram_tensor("x", [B_LOC, T, D], f32, kind="ExternalInput")
    for nm in ("Wk", "Wv", "Wq", "Wo"):
        io[nm + "_w"] = nc.dram_tensor(nm + "_w", [D, D], f32, kind="ExternalInput")
        io[nm + "_b"] = nc.dram_tensor(nm + "_b", [D], f32, kind="ExternalInput")
    io["w"] = nc.dram_tensor("w", [T, T], f32, kind="ExternalInput")
    io["out"] = nc.dram_tensor("out", [B_LOC, T, D], f32, kind="ExternalOutput")

    _emit(nc, io, tile, mybir, masks)
    nc.compile()
    return nc


def _emit(nc, io, tile, mybir, masks):
    f32 = mybir.dt.float32
    bf16 = mybir.dt.bfloat16
    Alu = mybir.AluOpType
    Act = mybir.ActivationFunctionType
    x, w, out = io["x"], io["w"], io["out"]

    with tile.TileContext(nc) as tc:
      try:
        with tc.tile_pool(name="sb", bufs=1) as sb, \
             tc.tile_pool(name="ps", bufs=1, space="PSUM") as ps, \
             tc.tile_pool(name="dram", bufs=1, space="DRAM") as dram:

            # ---------------- dummy collective to warm the CC path ---------
            dwarm_in = dram.tile([1, 32], f32)
            dwarm_out = dram.tile([1, 32], f32)
            zt = sb.tile([1, 32], f32, tag="zt")
            nc.gpsimd.memset(zt[:], 0.0)
            nc.sync.dma_start(dwarm_in[:], zt[:])
            nc.gpsimd.collective_compute(
                "AllReduce", Alu.max, replica_groups=[list(range(N_CORES))],
                ins=[dwarm_in.opt()], outs=[dwarm_out.opt()])

            # ---------------- constants ----------------------------------
            ident = sb.tile([P, P], f32, tag="ident")
            masks.make_identity(nc, ident[:])
            identb = sb.tile([P, P], bf16, tag="identb")
            masks.make_identity(nc, identb[:])
            ones_row = sb.tile([1, P], f32, tag="ones_row")
            nc.gpsimd.memset(ones_row[:], 1.0)

            # qb/vb as per-partition columns [128, ND]
            qb_col = sb.tile([P, ND], f32, tag="qb_col")
            nc.sync.dma_start(qb_col[:],
                              io["Wq_b"].ap().rearrange("(a b) -> b a", b=P))
            vb_col = sb.tile([P, ND], f32, tag="vb_col")
            nc.sync.dma_start(vb_col[:],
                              io["Wv_b"].ap().rearrange("(a b) -> b a", b=P))

            # broadcast biases along partitions via K=1 fp32 matmul
            bias_bcast = {}
            for nm, key in (("o", "Wo_b"),):
                brow = sb.tile([1, D], f32, tag="brow", name=f"brow_{nm}")
                nc.sync.dma_start(brow[:],
                                  io[key].ap().rearrange("(a b) -> a b", a=1))
                bps = ps.tile([P, D], f32, tag="sm", bufs=3, name=f"bps_{nm}")
                nc.tensor.matmul(bps[:], ones_row[:], brow[:])
                bc = sb.tile([P, D], f32, tag=f"bias_{nm}", name=f"bias_{nm}")
                nc.vector.tensor_copy(bc[:], bps[:])
                bias_bcast[nm] = bc

            # ---------------- weight transposes ---------------------------
            # WT[x][j] is a [128 (in-dim tile j), D (out-dim)] bf16 tile.
            WT = {}

            def prep_weight(nm, key):
                WT[nm] = [sb.tile([P, D], bf16, tag=f"WT_{nm}{j}",
                                  name=f"WT_{nm}{j}") for j in range(NE)]
                for i in range(ND):  # row tile of the natural weight
                    wload = sb.tile([P, D], f32, tag="wload", bufs=2)
                    nc.sync.dma_start(wload[:], io[key][i * P:(i + 1) * P, :])
                    wb16 = sb.tile([P, D], bf16, tag="wb16", bufs=3)
                    nc.vector.tensor_copy(wb16[:], wload[:])
                    for j in range(NE):
                        pt = ps.tile([P, P], bf16, tag="att", bufs=5)
                        nc.tensor.transpose(pt[:], wb16[:, j * P:(j + 1) * P],
                                            identb[:])
                        nc.scalar.copy(WT[nm][j][:, i * P:(i + 1) * P],
                                       pt[:])

            prep_weight("k", "Wk_w")
            prep_weight("v", "Wv_w")
            prep_weight("q", "Wq_w")
            prep_weight("o", "Wo_w")
            if _PHASE < 1:
                raise _PhaseStop()

            # ---------------- phase A1: xT and K per batch, local max -----
            # xT/YtT/sigT share one slot pool (lifetimes interleave).
            xT = [[sb.tile([P, T], bf16, tag="bigT", bufs=22, name=f"xT{b}_{e}")
                   for e in range(NE)] for b in range(B_LOC)]
            K_t = [[None] * NT for _ in range(B_LOC)]
            M_loc = sb.tile([P, NT * D], bf16, tag="Mshare", bufs=1)

            for b in range(B_LOC):
                for ti in range(NT):
                    xf = sb.tile([P, D], f32, tag="xf", bufs=2)
                    nc.sync.dma_start(xf[:], x[b, ti * P:(ti + 1) * P, :])
                    xb16 = sb.tile([P, D], bf16, tag="xb16", bufs=3)
                    nc.vector.tensor_copy(xb16[:], xf[:])
                    for ej in range(NE):
                        pt = ps.tile([P, P], bf16, tag="att", bufs=5)
                        nc.tensor.transpose(pt[:], xb16[:, ej * P:(ej + 1) * P],
                                            identb[:])
                        nc.scalar.copy(
                            xT[b][ej][:, ti * P:(ti + 1) * P], pt[:])
                for ti in range(NT):
                    kacc = ps.tile([P, D], f32, tag="sm", bufs=3)
                    for ej in range(NE):
                        nc.tensor.matmul(kacc[:],
                                         xT[b][ej][:, ti * P:(ti + 1) * P],
                                         WT["k"][ej][:],
                                         start=(ej == 0), stop=(ej == NE - 1))
                    ksb = sb.tile([P, D], bf16, tag="kd", bufs=33)
                    nc.vector.tensor_copy(ksb[:], kacc[:])
                    K_t[b][ti] = ksb
            for ti in range(NT):
                m01 = sb.tile([P, D], bf16, tag="mtree", bufs=4)
                nc.vector.tensor_tensor(m01[:], K_t[0][ti][:], K_t[1][ti][:],
                                        op=Alu.max)
                m23 = sb.tile([P, D], bf16, tag="mtree", bufs=4)
                nc.vector.tensor_tensor(m23[:], K_t[2][ti][:], K_t[3][ti][:],
                                        op=Alu.max)
                nc.vector.tensor_tensor(M_loc[:, ti * D:(ti + 1) * D],
                                        m01[:], m23[:], op=Alu.max)

            # ---------------- AllReduce(max) of the local K max ------------
            ar_in = dram.tile([P, NT * D], bf16)
            ar_out = dram.tile([P, NT * D], bf16)
            nc.sync.dma_start(ar_in[:], M_loc[:])
            nc.gpsimd.collective_compute(
                "AllReduce", Alu.max, replica_groups=[list(range(N_CORES))],
                ins=[ar_in.opt()], outs=[ar_out.opt()])
            M = sb.tile([P, NT * D], bf16, tag="Mshare", bufs=1)
            nc.sync.dma_start(M[:], ar_out[:])
            expnegM = sb.tile([P, NT * D], bf16, tag="expnegM")
            nc.scalar.activation(expnegM[:], M[:], Act.Exp, scale=-1.0)

            # ---------------- exp_w and its transpose ----------------------
            expwT = [sb.tile([P, T], bf16, tag=f"expwT{s}", name=f"expwT{s}")
                     for s in range(NT)]
            for wt in range(NT):
                wf = sb.tile([P, T], f32, tag="wf", bufs=1)
                nc.sync.dma_start(wf[:], w[wt * P:(wt + 1) * P, :])
                nrm = sb.tile([P, 1], f32, tag="nrm", bufs=2)
                nc.vector.reduce_max(nrm[:], wf[:], axis=mybir.AxisListType.X,
                                     negate=True)
                ew = sb.tile([P, T], bf16, tag="ew", bufs=2)
                nc.scalar.activation(ew[:], wf[:], Act.Exp, bias=nrm[:])
                for st in range(NT):
                    pt = ps.tile([P, P], bf16, tag="att", bufs=5)
                    nc.tensor.transpose(pt[:], ew[:, st * P:(st + 1) * P],
                                        identb[:])
                    nc.vector.tensor_copy(expwT[st][:, wt * P:(wt + 1) * P],
                                          pt[:])


            if _PHASE < 2:
                raise _PhaseStop()

            # ---------------- phase A2 (hides the AllReduce): V, sigT ------
            V_t = [[None] * NT for _ in range(B_LOC)]
            sigT = [[None] * ND for _ in range(B_LOC)]
            for b in range(B_LOC):
                for ti in range(NT):
                    vacc = ps.tile([P, D], f32, tag="sm", bufs=3)
                    for ej in range(NE):
                        nc.tensor.matmul(vacc[:],
                                         xT[b][ej][:, ti * P:(ti + 1) * P],
                                         WT["v"][ej][:],
                                         start=(ej == 0), stop=(ej == NE - 1))
                    vsb = sb.tile([P, D], bf16, tag="vd", bufs=33)
                    nc.vector.tensor_copy(vsb[:], vacc[:])
                    V_t[b][ti] = vsb
                for dj in range(ND):
                    sg = sb.tile([P, T], bf16, tag="bigT", bufs=22,
                                 name=f"sigT{b}_{dj}")
                    sigT[b][dj] = sg
                    for th in range(2):
                        qacc = ps.tile([P, D], f32, tag="sm", bufs=3)
                        for ej in range(NE):
                            nc.tensor.matmul(
                                qacc[:],
                                WT["q"][ej][:, dj * P:(dj + 1) * P],
                                xT[b][ej][:, th * D:(th + 1) * D],
                                start=(ej == 0), stop=(ej == NE - 1))
                        nc.scalar.activation(sg[:, th * D:(th + 1) * D],
                                             qacc[:], Act.Sigmoid,
                                             bias=qb_col[:, dj:dj + 1])
            if _PHASE < 3:
                raise _PhaseStop()

            if _PHASE < 4:
                raise _PhaseStop()

            # ---------------- phase B: attention + output ------------------
            for b in range(B_LOC):
                EK = [None] * NT
                U = [None] * NT
                for ti in range(NT):
                    eK = sb.tile([P, D], bf16, tag="sub", bufs=3)
                    nc.scalar.activation(eK[:], K_t[b][ti][:], Act.Exp)
                    ek = sb.tile([P, D], bf16, tag="kd", bufs=33)
                    nc.vector.tensor_tensor(ek[:], eK[:],
                                            expnegM[:, ti * D:(ti + 1) * D],
                                            op=Alu.mult)
                    EK[ti] = ek
                    u = sb.tile([P, D], bf16, tag="vd", bufs=33)
                    nc.vector.tensor_tensor(u[:], ek[:], V_t[b][ti][:],
                                            op=Alu.mult)
                    U[ti] = u
                if _PHASE < 5:
                    continue
                YtT = [None] * ND
                for dj in range(ND):
                    accs = [ps.tile([P, D], f32, tag="att", bufs=5,
                                    name=f"att{i}") for i in range(4)]
                    for si in range(NT):
                        first, last = (si == 0), (si == NT - 1)
                        usl = U[si][:, dj * P:(dj + 1) * P]
                        eksl = EK[si][:, dj * P:(dj + 1) * P]
                        for th in range(2):
                            nc.tensor.matmul(accs[th][:], usl,
                                             expwT[si][:, th * D:(th + 1) * D],
                                             start=first, stop=last)
                        for th in range(2):
                            nc.tensor.matmul(accs[2 + th][:], eksl,
                                             expwT[si][:, th * D:(th + 1) * D],
                                             start=first, stop=last)
                    yt = sb.tile([P, T], bf16, tag="bigT", bufs=22,
                                 name=f"ytT{b}_{dj}")
                    YtT[dj] = yt
                    for th in range(2):
                        rec = sb.tile([P, D], f32, tag="rec", bufs=2)
                        nc.vector.reciprocal_approx_fast(rec[:], accs[2 + th][:])
                        q = sb.tile([P, D], f32, tag="q", bufs=2)
                        nc.vector.tensor_tensor(q[:], accs[th][:], rec[:],
                                                op=Alu.mult)
                        nc.vector.scalar_tensor_tensor(
                            yt[:, th * D:(th + 1) * D], q[:],
                            vb_col[:, dj:dj + 1],
                            sigT[b][dj][:, th * D:(th + 1) * D],
                            op0=Alu.add, op1=Alu.mult)
                if _PHASE < 6:
                    continue
                for ti in range(NT):
                    oacc = ps.tile([P, D], f32, tag="sm", bufs=3)
                    for dj in range(ND):
                        nc.tensor.matmul(oacc[:],
                                         YtT[dj][:, ti * P:(ti + 1) * P],
                                         WT["o"][dj][:],
                                         start=(dj == 0), stop=(dj == ND - 1))
                    osb = sb.tile([P, D], f32, tag="osb", bufs=3)
                    nc.vector.tensor_tensor(osb[:], oacc[:], bias_bcast["o"][:],
                                            op=Alu.add)
                    nc.sync.dma_start(out[b, ti * P:(ti + 1) * P, :], osb[:])
      except _PhaseStop:
        pass


def _get_compiled():
    global _CACHED
    if _CACHED is None:
        _CACHED = _build()
    return _CACHED


def kernel(**inputs):
    from concourse.bass_utils import run_bass_kernel_spmd

    nc = _get_compiled()
    rep = {k: np.ascontiguousarray(inputs[k], dtype=np.float32)
           for k in ("Wk_w", "Wk_b", "Wv_w", "Wv_b", "Wq_w", "Wq_b",
                     "w", "Wo_w", "Wo_b")}
    xfull = np.ascontiguousarray(inputs["x"], dtype=np.float32)
    in_maps = []
    for c in range(N_CORES):
        m = dict(rep)
        m["x"] = np.ascontiguousarray(xfull[c * B_LOC:(c + 1) * B_LOC])
        in_maps.append(m)
    res = run_bass_kernel_spmd(nc, in_maps, core_ids=list(range(N_CORES)))
    return np.concatenate([res.results[c]["out"] for c in range(N_CORES)],
                          axis=0).astype(np.float32)


# revision 24
# speedup vs baseline: 1.0211x; 1.0211x over previous
"""AFT-Full attention kernel for 8 Trainium2 NeuronCores.

Data-parallel over batch B=32 (4 batches per core). The cross-batch max
of K (used for exp stabilization) is computed with an AllReduce(max)
collective over the 8 cores, overlapped with the V/Q projections.

All matmuls run in bf16 with fp32 PSUM accumulation. Layout strategy:
  - x is transposed per batch (PE transposes) to xT[e,t] so the feature
    contraction of the K/V/Q projections sits on the partition axis.
  - K, V are produced in natural [t,d] layout; Q is produced transposed
    (QT[d,t]) and immediately turned into sigmoid(Q) (sigT).
  - num/den are computed transposed (numT[d,t] = U/expK contracted
    against exp_wT[s,t]) so the output projection consumes YtT[d,t]
    directly and writes out[t,e] in natural layout for the store.
"""
import os
import sys

sys.path.insert(0, '/opt/trn_rl_repo')
import numpy as np

B, T, D = 32, 1024, 512
N_CORES = 8
B_LOC = B // N_CORES           # 4 batches per core
NT = T // 128                  # 8 t-tiles
ND = D // 128                  # 4 d-tiles
NE = D // 128                  # 4 e-tiles
P = 128

_PHASE = int(os.environ.get("KPHASE", "9"))
_CACHED = None


class _PhaseStop(Exception):
    pass


def _build():
    from concourse import bacc, mybir, tile, masks

    f32 = mybir.dt.float32

    nc = bacc.Bacc("TRN2", target_bir_lowering=False, debug=False,
                   num_devices=N_CORES)

    io = {}
    io["x"] = nc.dram_tensor("x", [B_LOC, T, D], f32, kind="ExternalInput")
    for nm in ("Wk", "Wv", "Wq", "Wo"):
        io[nm + "_w"] = nc.dram_tensor(nm + "_w", [D, D], f32, kind="ExternalInput")
        io[nm + "_b"] = nc.dram_tensor(nm + "_b", [D], f32, kind="ExternalInput")
    io["w"] = nc.dram_tensor("w", [T, T], f32, kind="ExternalInput")
    io["out"] = nc.dram_tensor("out", [B_LOC, T, D], f32, kind="ExternalOutput")

    _emit(nc, io, tile, mybir, masks)
    nc.compile()
    return nc


def _emit(nc, io, tile, mybir, masks):
    f32 = mybir.dt.float32
    bf16 = mybir.dt.bfloat16
    Alu = mybir.AluOpType
    Act = mybir.ActivationFunctionType
    x, w, out = io["x"], io["w"], io["out"]

    with tile.TileContext(nc) as tc:
      try:
        with tc.tile_pool(name="sb", bufs=1) as sb, \
             tc.tile_pool(name="ps", bufs=1, space="PSUM") as ps, \
             tc.tile_pool(name="dram", bufs=1, space="DRAM") as dram:

            # ---------------- dummy collective to warm the CC path ---------
            dwarm_in = dram.tile([1, 32], f32)
            dwarm_out = dram.tile([1, 32], f32)
            zt = sb.tile([1, 32], f32, tag="zt")
            nc.gpsimd.memset(zt[:], 0.0)
            nc.sync.dma_start(dwarm_in[:], zt[:])
            nc.gpsimd.collective_compute(
                "AllReduce", Alu.max, replica_groups=[list(range(N_CORES))],
                ins=[dwarm_in.opt()], outs=[dwarm_out.opt()])

            # ---------------- constants ----------------------------------
            ident = sb.tile([P, P], f32, tag="ident")
            masks.make_identity(nc, ident[:])
            identb = sb.tile([P, P], bf16, tag="identb")
            masks.make_identity(nc, identb[:])
            ones_row = sb.tile([1, P], f32, tag="ones_row")
            nc.gpsimd.memset(ones_row[:], 1.0)

            # qb/vb as per-partition columns [128, ND]
            qb_col = sb.tile([P, ND], f32, tag="qb_col")
            nc.sync.dma_start(qb_col[:],
                              io["Wq_b"].ap().rearrange("(a b) -> b a", b=P))
            vb_col = sb.tile([P, ND], f32, tag="vb_col")
            nc.sync.dma_start(vb_col[:],
                              io["Wv_b"].ap().rearrange("(a b) -> b a", b=P))

            # broadcast biases along partitions via K=1 fp32 matmul
            bias_bcast = {}
            for nm, key in (("o", "Wo_b"),):
                brow = sb.tile([1, D], f32, tag="brow", name=f"brow_{nm}")
                nc.sync.dma_start(brow[:],
                                  io[key].ap().rearrange("(a b) -> a b", a=1))
                bps = ps.tile([P, D], f32, tag="sm", bufs=3, name=f"bps_{nm}")
                nc.tensor.matmul(bps[:], ones_row[:], brow[:])
                bc = sb.tile([P, D], f32, tag=f"bias_{nm}", name=f"bias_{nm}")
                nc.vector.tensor_copy(bc[:], bps[:])
                bias_bcast[nm] = bc

            # ---------------- weight transposes ---------------------------
            # WT[x][j] is a [128 (in-dim tile j), D (out-dim)] bf16 tile.
            WT = {}

            def prep_weight(nm, key):
                WT[nm] = [sb.tile([P, D], bf16, tag=f"WT_{nm}{j}",
                                  name=f"WT_{nm}{j}") for j in range(NE)]
                for i in range(ND):  # row tile of the natural weight
                    wload = sb.tile([P, D], f32, tag="wload", bufs=3)
                    nc.sync.dma_start(wload[:], io[key][i * P:(i + 1) * P, :])
                    for j in range(NE):
                        pt = ps.tile([P, P], f32, tag="att", bufs=5)
                        nc.tensor.transpose(pt[:], wload[:, j * P:(j + 1) * P],
                                            ident[:])
                        nc.scalar.copy(WT[nm][j][:, i * P:(i + 1) * P],
                                       pt[:])

            prep_weight("k", "Wk_w")
            prep_weight("v", "Wv_w")
            prep_weight("q", "Wq_w")
            prep_weight("o", "Wo_w")
            if _PHASE < 1:
                raise _PhaseStop()

            # ---------------- phase A1: xT and K per batch, local max -----
            # xT/YtT/sigT share one slot pool (lifetimes interleave).
            xT = [[sb.tile([P, T], bf16, tag="bigT", bufs=22, name=f"xT{b}_{e}")
                   for e in range(NE)] for b in range(B_LOC)]
            K_t = [[None] * NT for _ in range(B_LOC)]
            M_loc = sb.tile([P, NT * D], bf16, tag="Mshare", bufs=1)

            for b in range(B_LOC):
                for ti in range(NT):
                    xf = sb.tile([P, D], f32, tag="xf", bufs=3)
                    nc.sync.dma_start(xf[:], x[b, ti * P:(ti + 1) * P, :])
                    for ej in range(NE):
                        pt = ps.tile([P, P], f32, tag="att", bufs=5)
                        nc.tensor.transpose(pt[:], xf[:, ej * P:(ej + 1) * P],
                                            ident[:])
                        nc.scalar.copy(
                            xT[b][ej][:, ti * P:(ti + 1) * P], pt[:])
                for ti in range(NT):
                    kacc = ps.tile([P, D], f32, tag="sm", bufs=3)
                    for ej in range(NE):
                        nc.tensor.matmul(kacc[:],
                                         xT[b][ej][:, ti * P:(ti + 1) * P],
                                         WT["k"][ej][:],
                                         start=(ej == 0), stop=(ej == NE - 1))
                    ksb = sb.tile([P, D], bf16, tag="kd", bufs=33)
                    nc.vector.tensor_copy(ksb[:], kacc[:])
                    K_t[b][ti] = ksb
            for ti in range(NT):
                m01 = sb.tile([P, D], bf16, tag="mtree", bufs=4)
                nc.vector.tensor_tensor(m01[:], K_t[0][ti][:], K_t[1][ti][:],
                                        op=Alu.max)
                m23 = sb.tile([P, D], bf16, tag="mtree", bufs=4)
                nc.vector.tensor_tensor(m23[:], K_t[2][ti][:], K_t[3][ti][:],
                                        op=Alu.max)
                nc.vector.tensor_tensor(M_loc[:, ti * D:(ti + 1) * D],
                                        m01[:], m23[:], op=Alu.max)

            # ---------------- AllReduce(max) of the local K max ------------
            # Split in two halves so phase B can start on half 0 early.
            H = NT * D // 2
            M = sb.tile([P, NT * D], bf16, tag="Mshare", bufs=1)
            for h in range(2):
                ar_in = dram.tile([P, H], bf16, name=f"ar_in{h}")
                ar_out = dram.tile([P, H], bf16, name=f"ar_out{h}")
                nc.sync.dma_start(ar_in[:], M_loc[:, h * H:(h + 1) * H])
                nc.gpsimd.collective_compute(
                    "AllReduce", Alu.max,
                    replica_groups=[list(range(N_CORES))],
                    ins=[ar_in.opt()], outs=[ar_out.opt()])
                nc.sync.dma_start(M[:, h * H:(h + 1) * H], ar_out[:])
            expnegM = sb.tile([P, NT * D], bf16, tag="expnegM")
            for h in range(2):
                nc.scalar.activation(expnegM[:, h * H:(h + 1) * H],
                                     M[:, h * H:(h + 1) * H], Act.Exp,
                                     scale=-1.0)

            # ---------------- exp_w and its transpose ----------------------
            expwT = [sb.tile([P, T], bf16, tag=f"expwT{s}", name=f"expwT{s}")
                     for s in range(NT)]
            for wt in range(NT):
                wf = sb.tile([P, T], f32, tag="wf", bufs=1)
                nc.sync.dma_start(wf[:], w[wt * P:(wt + 1) * P, :])
                nrm = sb.tile([P, 1], f32, tag="nrm", bufs=2)
                nc.vector.reduce_max(nrm[:], wf[:], axis=mybir.AxisListType.X,
                                     negate=True)
                ew = sb.tile([P, T], bf16, tag="ew", bufs=2)
                nc.scalar.activation(ew[:], wf[:], Act.Exp, bias=nrm[:])
                for st in range(NT):
                    pt = ps.tile([P, P], bf16, tag="att", bufs=5)
                    nc.tensor.transpose(pt[:], ew[:, st * P:(st + 1) * P],
                                        identb[:])
                    nc.vector.tensor_copy(expwT[st][:, wt * P:(wt + 1) * P],
                                          pt[:])


            if _PHASE < 2:
                raise _PhaseStop()

            # ---------------- phase A2 (hides the AllReduce): V, sigT ------
            V_t = [[None] * NT for _ in range(B_LOC)]
            sigT = [[None] * ND for _ in range(B_LOC)]
            for b in range(B_LOC):
                for ti in range(NT):
                    vacc = ps.tile([P, D], f32, tag="sm", bufs=3)
                    for ej in range(NE):
                        nc.tensor.matmul(vacc[:],
                                         xT[b][ej][:, ti * P:(ti + 1) * P],
                                         WT["v"][ej][:],
                                         start=(ej == 0), stop=(ej == NE - 1))
                    vsb = sb.tile([P, D], bf16, tag="vd", bufs=33)
                    nc.vector.tensor_copy(vsb[:], vacc[:])
                    V_t[b][ti] = vsb
                for dj in range(ND):
                    sg = sb.tile([P, T], bf16, tag="bigT", bufs=22,
                                 name=f"sigT{b}_{dj}")
                    sigT[b][dj] = sg
                    for th in range(2):
                        qacc = ps.tile([P, D], f32, tag="sm", bufs=3)
                        for ej in range(NE):
                            nc.tensor.matmul(
                                qacc[:],
                                WT["q"][ej][:, dj * P:(dj + 1) * P],
                                xT[b][ej][:, th * D:(th + 1) * D],
                                start=(ej == 0), stop=(ej == NE - 1))
                        nc.scalar.activation(sg[:, th * D:(th + 1) * D],
                                             qacc[:], Act.Sigmoid,
                                             bias=qb_col[:, dj:dj + 1])
            if _PHASE < 3:
                raise _PhaseStop()

            if _PHASE < 4:
                raise _PhaseStop()

            # ---------------- phase B: attention + output ------------------
            for b in range(B_LOC):
                EK = [None] * NT
                U = [None] * NT
                for ti in range(NT):
                    eK = sb.tile([P, D], bf16, tag="sub", bufs=3)
                    nc.scalar.activation(eK[:], K_t[b][ti][:], Act.Exp)
                    ek = sb.tile([P, D], bf16, tag="kd", bufs=33)
                    nc.vector.tensor_tensor(ek[:], eK[:],
                                            expnegM[:, ti * D:(ti + 1) * D],
                                            op=Alu.mult)
                    EK[ti] = ek
                    u = sb.tile([P, D], bf16, tag="vd", bufs=33)
                    nc.vector.tensor_tensor(u[:], ek[:], V_t[b][ti][:],
                                            op=Alu.mult)
                    U[ti] = u
                if _PHASE < 5:
                    continue
                YtT = [None] * ND
                for dj in range(ND):
                    accs = [ps.tile([P, D], f32, tag="att", bufs=5,
                                    name=f"att{i}") for i in range(4)]
                    for si in range(NT):
                        first, last = (si == 0), (si == NT - 1)
                        usl = U[si][:, dj * P:(dj + 1) * P]
                        eksl = EK[si][:, dj * P:(dj + 1) * P]
                        for th in range(2):
                            nc.tensor.matmul(accs[th][:], usl,
                                             expwT[si][:, th * D:(th + 1) * D],
                                             start=first, stop=last)
                        for th in range(2):
                            nc.tensor.matmul(accs[2 + th][:], eksl,
                                             expwT[si][:, th * D:(th + 1) * D],
                                             start=first, stop=last)
                    yt = sb.tile([P, T], bf16, tag="bigT", bufs=22,
                                 name=f"ytT{b}_{dj}")
                    YtT[dj] = yt
                    for th in range(2):
                        rec = sb.tile([P, D], f32, tag="rec", bufs=2)
                        nc.vector.reciprocal_approx_fast(rec[:], accs[2 + th][:])
                        q = sb.tile([P, D], f32, tag="q", bufs=2)
                        nc.vector.tensor_tensor(q[:], accs[th][:], rec[:],
                                                op=Alu.mult)
                        nc.vector.scalar_tensor_tensor(
                            yt[:, th * D:(th + 1) * D], q[:],
                            vb_col[:, dj:dj + 1],
                            sigT[b][dj][:, th * D:(th + 1) * D],
                            op0=Alu.add, op1=Alu.mult)
                if _PHASE < 6:
                    continue
                for ti in range(NT):
                    oacc = ps.tile([P, D], f32, tag="sm", bufs=3)
                    for dj in range(ND):
                        nc.tensor.matmul(oacc[:],
                                         YtT[dj][:, ti * P:(ti + 1) * P],
                                         WT["o"][dj][:],
                                         start=(dj == 0), stop=(dj == ND - 1))
                    osb = sb.tile([P, D], f32, tag="osb", bufs=3)
                    nc.vector.tensor_tensor(osb[:], oacc[:], bias_bcast["o"][:],
                                            op=Alu.add)
                    nc.sync.dma_start(out[b, ti * P:(ti + 1) * P, :], osb[:])
      except _PhaseStop:
        pass


def _get_compiled():
    global _CACHED
    if _CACHED is None:
        _CACHED = _build()
    return _CACHED


def kernel(**inputs):
    from concourse.bass_utils import run_bass_kernel_spmd

    nc = _get_compiled()
    rep = {k: np.ascontiguousarray(inputs[k], dtype=np.float32)
           for k in ("Wk_w", "Wk_b", "Wv_w", "Wv_b", "Wq_w", "Wq_b",
                     "w", "Wo_w", "Wo_b")}
    xfull = np.ascontiguousarray(inputs["x"], dtype=np.float32)
    in_maps = []
    for c in range(N_CORES):
        m = dict(rep)
        m["x"] = np.ascontiguousarray(xfull[c * B_LOC:(c + 1) * B_LOC])
        in_maps.append(m)
    res = run_bass_kernel_spmd(nc, in_maps, core_ids=list(range(N_CORES)))
    return np.concatenate([res.results[c]["out"] for c in range(N_CORES)],
                          axis=0).astype(np.float32)
